# revision 10
# baseline (speedup 1.0000x reference)
"""MoE feed-forward (top-1 routing, capacity 640, swiglu experts) on 8 trn2 cores.

Strategy (expert-parallel, per the sharding hint):
  * Host: router matmul/softmax/argmax + capacity-slot assignment (index
    plumbing, ~0.1% of FLOPs), gathers tokens per expert and packs them
    into per-core "slots".  Default plan is 3 slots per core: the 8
    lightest experts ride whole in slot 0 (size = max of their counts),
    and the 8 heaviest are split into half-pieces, two cores each, in
    slots 1 and 2 (sizes = ceil(max/2) of their quartile).  That cuts the
    padded per-core token count versus the 2-slot pairing (every core
    runs the same program, so slot sizes are the elementwise max over
    cores); a small cost model picks 2-slot when the distribution favors
    it.
  * Device (Bass/Tile, per core): per slot, grouped GEMM  h = x @ W1  ->
    swiglu -> y = g @ W2.  Matmuls in bf16 with fp32 accumulate.  Tokens
    are the MOVING dim in both GEMMs, so slot sizes are token-granular.
    The first matmul is primed with tiny DMAs (first W1 k-tile + first x
    k-block) striped over the sync/vector/scalar queues so the tensor
    engine starts ~4us earlier than a whole-slot load would allow.  W1
    streams per feature-pair tile on the sync queue with a deep ring
    buffer (prefetches the next slot's W1 during this slot's GEMM2); W2
    for slot s+1 streams during slot s's GEMM2 on the scalar queue; y
    leaves as bf16 on scalar.
  * Host: scatter expert outputs back to token order, applying the
    combine gates (and b2 / dense fallback when nonzero) on the fly.
"""

import os
import sys

import numpy as np


def _ensure_concourse():
    try:
        import concourse.bass  # noqa: F401
    except Exception:
        for p in ("/opt/trn_rl_repo", "/root/.axon_site/_ro/trn_rl_repo"):
            if os.path.isdir(p) and p not in sys.path:
                sys.path.insert(0, p)
        import concourse.bass  # noqa: F401


# Problem constants (hardcoded per the task contract).
B, S, D, H, E = 4, 2048, 768, 3072, 16
N = B * S
C = 640  # capacity per expert (ceil(1.25 * N / E))
FALLBACK_W = 1.0
NCORES = 8
KD = D // 128  # 6 k-tiles for GEMM1 contraction
FB = (2 * H) // 128  # 48 feature blocks of GEMM1 output
FP = FB // 2  # 24 swiglu pairs == k-tiles of GEMM2 contraction
KH = H // 128  # 24

_NC_CACHE = {}  # ts tuple -> compiled Bass program
_WCACHE = {}  # weight reorder cache
LAST = None  # BassKernelResults of the most recent run (for profiling)


def _chunks(T, cap=512):
    """Split T tokens into <=cap moving-dim chunks, all >=212 when possible."""
    n = max(1, -(-T // cap))
    if T < 212 * n:
        n = max(1, -(-T // 512))
    base = T // n
    out, off = [], 0
    for i in range(n):
        w = base + (1 if i < T - base * n else 0)
        out.append((off, w))
        off += w
    return out


def _stream_cost(ts):
    """Model of the tensor-engine stream time (ns) for a slot-size tuple.

    Calibrated on hardware: each matmul covers its chunk width (0.4167
    ns/col at 2.4 GHz), is floored by the ~97 ns LDWEIGHTS shadow, and
    carries ~5.2 ns fixed overhead.  432 matmuls per chunk.
    """
    c = 0.0
    for T in ts:
        for _, cw in _chunks(T):
            c += 432.0 * (max(0.4167 * cw, 97.0) + 5.2)
    return c


def _build_nc(ts):
    """Per-core Bass program: len(ts) expert slots with ts[s] tokens each."""
    import concourse.bacc as bacc
    import concourse.mybir as mybir
    import concourse.tile as tile
    from contextlib import ExitStack

    f32 = mybir.dt.float32
    bf16 = mybir.dt.bfloat16
    AF = mybir.ActivationFunctionType
    ALU = mybir.AluOpType

    EL = len(ts)
    tot = sum(ts)
    offs = [sum(ts[:s]) for s in range(EL)]

    nc = bacc.Bacc("TRN2", target_bir_lowering=False)
    # Host-side layouts are pre-tiled so every DMA is 2D [128, contiguous].
    xt = nc.dram_tensor("xt", [128, KD * tot], bf16, kind="ExternalInput")
    w1r = nc.dram_tensor("w1r", [EL, FP, 128, 2 * KD * 128], bf16, kind="ExternalInput")
    w2t = nc.dram_tensor("w2t", [EL, 128, KH * D], bf16, kind="ExternalInput")
    b1t = nc.dram_tensor("b1t", [EL, 128, FB], f32, kind="ExternalInput")
    y = nc.dram_tensor("y", [D, tot], bf16, kind="ExternalOutput")

    with tile.TileContext(nc) as tc, ExitStack() as ctx:
        xp = ctx.enter_context(tc.tile_pool(name="xp", bufs=1))
        gp = ctx.enter_context(tc.tile_pool(name="gp", bufs=1))
        w2p = ctx.enter_context(tc.tile_pool(name="w2p", bufs=2))
        w1p = ctx.enter_context(tc.tile_pool(name="w1p", bufs=12))
        sap = ctx.enter_context(tc.tile_pool(name="sap", bufs=3))
        cst = ctx.enter_context(tc.tile_pool(name="cst", bufs=2))
        yp = ctx.enter_context(tc.tile_pool(name="yp", bufs=4))
        p1 = ctx.enter_context(tc.tile_pool(name="p1", bufs=3, space="PSUM"))
        p2 = ctx.enter_context(tc.tile_pool(name="p2", bufs=2, space="PSUM"))
        # DMA plan: every dma_start costs ~600ns of descriptor generation
        # on the issuing sequencer, so counts and queue placement matter.
        # Slot 0's x is primed one k-block at a time, alternating between
        # the sync and vector queues, so the first matmul only waits for
        # its own k-block + W1 k-tile.  Later slots' x rides the (idle)
        # vector queue; W1 streams on sync; W2/y on scalar; gpsimd (soft
        # DGE, slow ring) only carries late b1 tiles.
        xsb = xp.tile([128, KD * tot], bf16, tag="x")

        b1sb = [
            cst.tile([128, FB], f32, tag=f"b1_{s}", name=f"b1sb{s}")
            for s in range(EL)
        ]
        # later slots' b1 is not needed until much later; keep it off the
        # hot queues
        for s in range(1, EL):
            nc.gpsimd.dma_start(b1sb[s][:], b1t[s, :, :])

        gt = gp.tile([128, KH * tot], bf16, tag="g")

        w2sb = [None] * EL

        def w2_load(e, klo, khi):
            nc.scalar.dma_start(
                w2sb[e][:, klo * D : khi * D], w2t[e, :, klo * D : khi * D]
            )

        w2sb[0] = w2p.tile([128, KH * D], bf16, tag="w2", name="w2sb0")

        for e in range(EL):
            T = ts[e]
            xoff = KD * offs[e]
            goff = KH * offs[e]
            yoff = offs[e]
            chs = _chunks(T)

            # GEMM1 + swiglu: hT tiles [feat 128, tok chunk]
            for fp in range(FP):
                w1t = w1p.tile([128, 2 * KD * 128], bf16, tag="w1")
                if e == 0 and fp == 0:
                    # Startup: the scalar DMA queue is very slow for the
                    # first ~6us, so EVERYTHING the first feature-pair
                    # needs goes on sync, split into small descriptors in
                    # exact consumption-deadline order: (W1a k0, x k0),
                    # (W1a k1.., x k1..), W1b, b1.
                    T0 = ts[0]
                    nc.sync.dma_start(w1t[:, :128], w1r[e, fp, :, :128])
                    nc.sync.dma_start(xsb[:, :T0], xt[:, :T0])
                    nc.sync.dma_start(
                        w1t[:, 128 : KD * 128], w1r[e, fp, :, 128 : KD * 128]
                    )
                    nc.sync.dma_start(xsb[:, T0 : 2 * T0], xt[:, T0 : 2 * T0])
                    nc.sync.dma_start(w1t[:, KD * 128 :], w1r[e, fp, :, KD * 128 :])
                    nc.sync.dma_start(xsb[:, 2 * T0 : 3 * T0], xt[:, 2 * T0 : 3 * T0])
                    nc.sync.dma_start(b1sb[0][:], b1t[0, :, :])
                    nc.sync.dma_start(
                        xsb[:, 3 * T0 : KD * T0], xt[:, 3 * T0 : KD * T0]
                    )
                else:
                    nc.sync.dma_start(w1t[:], w1r[e, fp, :, :])
                w1a = w1t[:, : KD * 128]
                w1b = w1t[:, KD * 128 :]
                for coff, cw in chs:
                    pa = p1.tile([128, cw], f32, tag="pa")
                    pb = p1.tile([128, cw], f32, tag="pb")
                    for k in range(KD):
                        nc.tensor.matmul(
                            pa[:],
                            lhsT=w1a[:, k * 128 : (k + 1) * 128],
                            rhs=xsb[:, xoff + k * T + coff : xoff + k * T + coff + cw],
                            start=(k == 0),
                            stop=(k == KD - 1),
                        )
                    for k in range(KD):
                        nc.tensor.matmul(
                            pb[:],
                            lhsT=w1b[:, k * 128 : (k + 1) * 128],
                            rhs=xsb[:, xoff + k * T + coff : xoff + k * T + coff + cw],
                            start=(k == 0),
                            stop=(k == KD - 1),
                        )
                    sa = sap.tile([128, cw], f32, tag="sa")
                    # silu(a + b1_a)
                    nc.scalar.activation(
                        sa[:], pa[:], AF.Silu, bias=b1sb[e][:, fp : fp + 1], scale=1.0
                    )
                    # g = (b + b1_b) * silu(...)
                    nc.vector.scalar_tensor_tensor(
                        out=gt[:, goff + fp * T + coff : goff + fp * T + coff + cw],
                        in0=pb[:],
                        scalar=b1sb[e][:, FP + fp : FP + fp + 1],
                        in1=sa[:],
                        op0=ALU.add,
                        op1=ALU.mult,
                    )
                # deferred next-slot x load, off the startup bandwidth peak
                if fp == 6 and e + 1 < EL:
                    nxoff = KD * offs[e + 1]
                    nc.scalar.dma_start(
                        xsb[:, nxoff : nxoff + KD * ts[e + 1]],
                        xt[:, nxoff : nxoff + KD * ts[e + 1]],
                    )
                # slot 0's own W2: paced loads in the SECOND half of its
                # GEMM1, clear of the startup bandwidth crunch
                if e == 0 and fp % 2 == 1 and fp >= 9:
                    j = (fp - 9) // 2
                    w2_load(0, 3 * j, 3 * j + 3)

            # GEMM2: yT[d 128, tok chunk] = sum_k W2[h_k, d].T @ g[h_k, tok]
            # while it runs, stream the NEXT slot's W2 on scalar (the w2p
            # ring has 2 buffers) and let the w1p ring prefetch the next
            # slot's W1 on sync.
            nsteps = len(chs) * KD
            if e + 1 < EL:
                w2sb[e + 1] = w2p.tile(
                    [128, KH * D], bf16, tag="w2", name=f"w2sb{e + 1}"
                )
            step = 0
            for coff, cw in chs:
                for dt in range(KD):
                    pt = p2.tile([128, cw], f32, tag="p2")
                    for k in range(KH):
                        nc.tensor.matmul(
                            pt[:],
                            lhsT=w2sb[e][:, k * D + dt * 128 : k * D + (dt + 1) * 128],
                            rhs=gt[:, goff + k * T + coff : goff + k * T + coff + cw],
                            start=(k == 0),
                            stop=(k == KH - 1),
                        )
                    ysb = yp.tile([128, cw], bf16, tag="y")
                    nc.scalar.activation(ysb[:], pt[:], AF.Copy, bias=0.0, scale=1.0)
                    nc.scalar.dma_start(
                        y[dt * 128 : (dt + 1) * 128, yoff + coff : yoff + coff + cw],
                        ysb[:],
                    )
                    if e + 1 < EL:
                        klo = (12 * step) // nsteps * 2
                        khi = (12 * (step + 1)) // nsteps * 2
                        if khi > klo:
                            w2_load(e + 1, klo, khi)
                    step += 1
    nc.compile()
    return nc


def _get_nc(ts):
    nc = _NC_CACHE.get(ts)
    if nc is None:
        nc = _NC_CACHE[ts] = _build_nc(ts)
    return nc


def _reorder_weights(W1, W2, b1):
    key = (W1.__array_interface__["data"][0], W2.__array_interface__["data"][0])
    hit = _WCACHE.get(key)
    if hit is not None:
        return hit
    import ml_dtypes

    W1 = np.ascontiguousarray(W1, dtype=np.float32)
    W2 = np.ascontiguousarray(W2, dtype=np.float32)
    b1 = np.ascontiguousarray(b1, dtype=np.float32)
    # W1 [E, D, 2H] -> [E, FB, 128p(d within k), KD*128(f)]
    w1f = (
        W1.reshape(E, KD, 128, FB, 128)
        .transpose(0, 3, 2, 1, 4)
        .reshape(E, FB, 128, KD * 128)
        .astype(ml_dtypes.bfloat16)
    )
    # combine swiglu pair (fp, fp+FP) into one contiguous block per DMA
    w1r = np.ascontiguousarray(np.concatenate([w1f[:, :FP], w1f[:, FP:]], axis=-1))
    # W2 [E, H, D] -> [E, 128p(h within k), KH*D]
    w2t = np.ascontiguousarray(
        W2.reshape(E, KH, 128, D)
        .transpose(0, 2, 1, 3)
        .reshape(E, 128, KH * D)
        .astype(ml_dtypes.bfloat16)
    )
    # b1 [E, 2H] -> [E, 128, FB]
    b1t = np.ascontiguousarray(b1.reshape(E, FB, 128).transpose(0, 2, 1))
    out = (w1r, w2t, b1t)
    _WCACHE.clear()
    _WCACHE[key] = out
    return out


def _route(x_flat, Wr):
    logits = x_flat @ np.ascontiguousarray(Wr, dtype=np.float32)  # [N, E]
    lmax = logits.max(axis=-1, keepdims=True)
    p = np.exp(logits - lmax)
    gates = p / p.sum(axis=-1, keepdims=True)
    expert = np.argmax(gates, axis=-1)
    # slot = occurrence index of each token within its expert's queue
    order = np.argsort(expert, kind="stable")
    sorted_e = expert[order]
    starts = np.searchsorted(sorted_e, np.arange(E))
    within = np.arange(N) - starts[sorted_e]
    slot = np.empty(N, np.int64)
    slot[order] = within
    kept = slot < C
    top_idx = np.zeros((C, E), np.int32)
    valid = np.zeros((C, E), np.float32)
    tok = np.arange(N, dtype=np.int32)
    top_idx[slot[kept], expert[kept]] = tok[kept]
    valid[slot[kept], expert[kept]] = 1.0
    w_ce = gates[top_idx, np.arange(E)[None, :]].astype(np.float32) * valid  # [C, E]
    n_kept = np.minimum(np.bincount(expert, minlength=E), C)  # [E]
    return gates, expert, kept, top_idx, valid, w_ce, n_kept


def _r16(n):
    return max(16, int(n))


def _plan(n_kept):
    """Choose slot sizes + per-(core,slot) expert piece assignment.

    Returns (ts, cores) where cores[c] is a list of (expert, lo, hi)
    token-range pieces, one per slot.
    """
    order = np.argsort(-n_kept, kind="stable")
    # Plan A (2 slots): pair light+heavy; sizes = (8th largest, largest).
    assign2 = [(int(order[E - 1 - i]), int(order[i])) for i in range(NCORES)]
    ts2 = (
        _r16(max(n_kept[a] for a, _ in assign2)),
        _r16(max(n_kept[b] for _, b in assign2)),
    )
    cores2 = [
        [(a, 0, int(n_kept[a])), (b, 0, int(n_kept[b]))] for a, b in assign2
    ]
    # Plan B (3 slots): 8 smallest whole; quartiles 1,2 split in halves
    # across two cores each.
    G1, G2, rest = order[:4], order[4:8], order[8:]
    sC = _r16(n_kept[rest[0]])
    sA = _r16(-(-int(n_kept[G1[0]]) // 2))
    sB = _r16(-(-int(n_kept[G2[0]]) // 2))
    ts3 = (sC, sA, sB)
    cores3 = []
    for c in range(NCORES):
        ec = int(rest[c])
        slots = [(ec, 0, int(n_kept[ec]))]
        for G in (G1, G2):
            e = int(G[c % 4])
            n = int(n_kept[e])
            h = -(-n // 2)
            slots.append((e, 0, h) if c < 4 else (e, h, n))
        cores3.append(slots)
    if _stream_cost(ts3) < _stream_cost(ts2):
        return ts3, cores3
    return ts2, cores2


def kernel(x, Wr, W1, b1, W2, b2, W1f, b1f, W2f, b2f, _trace=False):
    global LAST
    _ensure_concourse()
    import ml_dtypes
    from concourse.bass_utils import run_bass_kernel_spmd

    x_flat = np.ascontiguousarray(np.asarray(x).reshape(N, D), dtype=np.float32)
    gates, expert, kept, top_idx, valid, w_ce, n_kept = _route(x_flat, np.asarray(Wr))
    w1r, w2t, b1t = _reorder_weights(np.asarray(W1), np.asarray(W2), np.asarray(b1))

    ts, cores = _plan(n_kept)

    nc = _get_nc(ts)
    in_maps = []
    for c in range(NCORES):
        pieces = cores[c]
        # gather + transpose tokens for each slot: [128, KD * T]
        xparts = []
        for s, (e, lo, hi) in enumerate(pieces):
            ids = top_idx[lo:hi, e]
            xg = np.zeros((ts[s], D), np.float32)
            xg[: len(ids)] = x_flat[ids]
            xparts.append(
                xg.reshape(ts[s], KD, 128).transpose(2, 1, 0).reshape(128, KD * ts[s])
            )
        xt_c = np.ascontiguousarray(
            np.concatenate(xparts, axis=1), dtype=ml_dtypes.bfloat16
        )
        el = [e for e, _, _ in pieces]
        in_maps.append(
            {
                "xt": xt_c,
                "w1r": np.ascontiguousarray(w1r[el]),
                "w2t": np.ascontiguousarray(w2t[el]),
                "b1t": np.ascontiguousarray(b1t[el]),
            }
        )
    res = run_bass_kernel_spmd(nc, in_maps, list(range(NCORES)), trace=_trace)
    LAST = res

    # Combine: scatter gate-weighted expert outputs back to token order.
    y_flat = np.zeros((N, D), np.float32)
    b2 = np.asarray(b2)
    add_b2 = bool(np.any(b2))
    offs = [sum(ts[:s]) for s in range(len(ts))]
    for c in range(NCORES):
        yc = np.asarray(res.results[c]["y"], dtype=np.float32)  # [D, tot]
        for s, (e, lo, hi) in enumerate(cores[c]):
            n = hi - lo
            ids = top_idx[lo:hi, e]
            off = offs[s]
            w = w_ce[lo:hi, e]
            y_flat[ids] = (yc[:, off : off + n] * w[None, :]).T
            if add_b2:
                y_flat[ids] += w[:, None] * b2[e]

    # Dense fallback for fully-dropped tokens (rare; none at typical loads).
    dropped = ~kept
    if np.any(dropped):
        xd = x_flat[dropped]
        hf = xd @ np.asarray(W1f) + np.asarray(b1f)
        gf = (hf[:, :H] / (1.0 + np.exp(-hf[:, :H]))) * hf[:, H:]
        y_flat[dropped] += FALLBACK_W * (gf @ np.asarray(W2f) + np.asarray(b2f))

    return y_flat.reshape(B, S, D)


# revision 12
# speedup vs baseline: 1.0101x; 1.0101x over previous
"""MoE feed-forward (top-1 routing, capacity 640, swiglu experts) on 8 trn2 cores.

Strategy (expert-parallel, per the sharding hint):
  * Host: router matmul/softmax/argmax + capacity-slot assignment (index
    plumbing, ~0.1% of FLOPs), gathers tokens per expert and packs them
    into per-core "slots".  Default plan is 3 slots per core: the 8
    lightest experts ride whole in slot 0 (size = max of their counts),
    and the 8 heaviest are split into half-pieces, two cores each, in
    slots 1 and 2 (sizes = ceil(max/2) of their quartile).  That cuts the
    padded per-core token count versus the 2-slot pairing (every core
    runs the same program, so slot sizes are the elementwise max over
    cores); a small cost model picks 2-slot when the distribution favors
    it.
  * Device (Bass/Tile, per core): per slot, grouped GEMM  h = x @ W1  ->
    swiglu -> y = g @ W2.  Matmuls in bf16 with fp32 accumulate.  Tokens
    are the MOVING dim in both GEMMs, so slot sizes are token-granular.
    The first matmul is primed with tiny DMAs (first W1 k-tile + first x
    k-block) striped over the sync/vector/scalar queues so the tensor
    engine starts ~4us earlier than a whole-slot load would allow.  W1
    streams per feature-pair tile on the sync queue with a deep ring
    buffer (prefetches the next slot's W1 during this slot's GEMM2); W2
    for slot s+1 streams during slot s's GEMM2 on the scalar queue; y
    leaves as bf16 on scalar.
  * Host: scatter expert outputs back to token order, applying the
    combine gates (and b2 / dense fallback when nonzero) on the fly.
"""

import os
import sys

import numpy as np


def _ensure_concourse():
    try:
        import concourse.bass  # noqa: F401
    except Exception:
        for p in ("/opt/trn_rl_repo", "/root/.axon_site/_ro/trn_rl_repo"):
            if os.path.isdir(p) and p not in sys.path:
                sys.path.insert(0, p)
        import concourse.bass  # noqa: F401


# Problem constants (hardcoded per the task contract).
B, S, D, H, E = 4, 2048, 768, 3072, 16
N = B * S
C = 640  # capacity per expert (ceil(1.25 * N / E))
FALLBACK_W = 1.0
NCORES = 8
KD = D // 128  # 6 k-tiles for GEMM1 contraction
FB = (2 * H) // 128  # 48 feature blocks of GEMM1 output
FP = FB // 2  # 24 swiglu pairs == k-tiles of GEMM2 contraction
KH = H // 128  # 24

_NC_CACHE = {}  # ts tuple -> compiled Bass program
_WCACHE = {}  # weight reorder cache
LAST = None  # BassKernelResults of the most recent run (for profiling)


def _chunks(T, cap=512):
    """Split T tokens into <=cap moving-dim chunks, all >=212 when possible."""
    n = max(1, -(-T // cap))
    if T < 212 * n:
        n = max(1, -(-T // 512))
    base = T // n
    out, off = [], 0
    for i in range(n):
        w = base + (1 if i < T - base * n else 0)
        out.append((off, w))
        off += w
    return out


def _stream_cost(ts):
    """Model of the tensor-engine stream time (ns) for a slot-size tuple.

    Calibrated on hardware: each matmul covers its chunk width (0.4167
    ns/col at 2.4 GHz), is floored by the ~97 ns LDWEIGHTS shadow, and
    carries ~5.2 ns fixed overhead.  432 matmuls per chunk.
    """
    c = 0.0
    for T in ts:
        for _, cw in _chunks(T):
            c += 432.0 * (max(0.4167 * cw, 97.0) + 5.2)
    return c


def _build_nc(ts):
    """Per-core Bass program: len(ts) expert slots with ts[s] tokens each."""
    import concourse.bacc as bacc
    import concourse.mybir as mybir
    import concourse.tile as tile
    from contextlib import ExitStack

    f32 = mybir.dt.float32
    bf16 = mybir.dt.bfloat16
    AF = mybir.ActivationFunctionType
    ALU = mybir.AluOpType

    EL = len(ts)
    tot = sum(ts)
    offs = [sum(ts[:s]) for s in range(EL)]

    nc = bacc.Bacc("TRN2", target_bir_lowering=False)
    # Host-side layouts are pre-tiled so every DMA is 2D [128, contiguous].
    xt = nc.dram_tensor("xt", [128, KD * tot], bf16, kind="ExternalInput")
    w1r = nc.dram_tensor("w1r", [EL, FP, 128, 2 * KD * 128], bf16, kind="ExternalInput")
    w2t = nc.dram_tensor("w2t", [EL, 128, KH * D], bf16, kind="ExternalInput")
    b1t = nc.dram_tensor("b1t", [EL, 128, FB], f32, kind="ExternalInput")
    y = nc.dram_tensor("y", [D, tot], bf16, kind="ExternalOutput")

    with tile.TileContext(nc) as tc, ExitStack() as ctx:
        xp = ctx.enter_context(tc.tile_pool(name="xp", bufs=1))
        gp = ctx.enter_context(tc.tile_pool(name="gp", bufs=1))
        w2p = ctx.enter_context(tc.tile_pool(name="w2p", bufs=2))
        w1p = ctx.enter_context(tc.tile_pool(name="w1p", bufs=12))
        sap = ctx.enter_context(tc.tile_pool(name="sap", bufs=3))
        cst = ctx.enter_context(tc.tile_pool(name="cst", bufs=2))
        yp = ctx.enter_context(tc.tile_pool(name="yp", bufs=4))
        p1 = ctx.enter_context(tc.tile_pool(name="p1", bufs=3, space="PSUM"))
        p2 = ctx.enter_context(tc.tile_pool(name="p2", bufs=2, space="PSUM"))
        # DMA plan: every dma_start costs ~600ns of descriptor generation
        # on the issuing sequencer, so counts and queue placement matter.
        # Slot 0's x is primed one k-block at a time, alternating between
        # the sync and vector queues, so the first matmul only waits for
        # its own k-block + W1 k-tile.  Later slots' x rides the (idle)
        # vector queue; W1 streams on sync; W2/y on scalar; gpsimd (soft
        # DGE, slow ring) only carries late b1 tiles.
        xsb = xp.tile([128, KD * tot], bf16, tag="x")

        b1sb = [
            cst.tile([128, FB], f32, tag=f"b1_{s}", name=f"b1sb{s}")
            for s in range(EL)
        ]
        # later slots' b1 is not needed until much later; keep it off the
        # hot queues
        for s in range(1, EL):
            nc.gpsimd.dma_start(b1sb[s][:], b1t[s, :, :])

        gt = gp.tile([128, KH * tot], bf16, tag="g")

        w2sb = [None] * EL

        def w2_load(e, klo, khi):
            nc.scalar.dma_start(
                w2sb[e][:, klo * D : khi * D], w2t[e, :, klo * D : khi * D]
            )

        w2sb[0] = w2p.tile([128, KH * D], bf16, tag="w2", name="w2sb0")

        for e in range(EL):
            T = ts[e]
            xoff = KD * offs[e]
            goff = KH * offs[e]
            yoff = offs[e]
            chs = _chunks(T)

            # GEMM1 + swiglu: hT tiles [feat 128, tok chunk]
            for fp in range(FP):
                w1t = w1p.tile([128, 2 * KD * 128], bf16, tag="w1")
                if e == 0 and fp == 0:
                    # Startup is pure-bandwidth-bound: one big x descriptor
                    # on sync (stripes well) in parallel with fp0's W1 on
                    # scalar is the proven-fastest arrangement; the first
                    # matmul waits ~12.6us and the stream then runs with
                    # zero stalls.  (Fine-grained priming starts matmuls
                    # earlier but loses more to queue head-of-line
                    # blocking.)
                    T0 = ts[0]
                    nc.sync.dma_start(xsb[:, : KD * T0], xt[:, : KD * T0])
                    nc.scalar.dma_start(w1t[:, : KD * 128], w1r[e, fp, :, : KD * 128])
                    nc.scalar.dma_start(b1sb[0][:], b1t[0, :, :])
                    nc.scalar.dma_start(w1t[:, KD * 128 :], w1r[e, fp, :, KD * 128 :])
                else:
                    nc.sync.dma_start(w1t[:], w1r[e, fp, :, :])
                w1a = w1t[:, : KD * 128]
                w1b = w1t[:, KD * 128 :]
                for coff, cw in chs:
                    pa = p1.tile([128, cw], f32, tag="pa")
                    pb = p1.tile([128, cw], f32, tag="pb")
                    for k in range(KD):
                        nc.tensor.matmul(
                            pa[:],
                            lhsT=w1a[:, k * 128 : (k + 1) * 128],
                            rhs=xsb[:, xoff + k * T + coff : xoff + k * T + coff + cw],
                            start=(k == 0),
                            stop=(k == KD - 1),
                        )
                    for k in range(KD):
                        nc.tensor.matmul(
                            pb[:],
                            lhsT=w1b[:, k * 128 : (k + 1) * 128],
                            rhs=xsb[:, xoff + k * T + coff : xoff + k * T + coff + cw],
                            start=(k == 0),
                            stop=(k == KD - 1),
                        )
                    sa = sap.tile([128, cw], f32, tag="sa")
                    # silu(a + b1_a)
                    nc.scalar.activation(
                        sa[:], pa[:], AF.Silu, bias=b1sb[e][:, fp : fp + 1], scale=1.0
                    )
                    # g = (b + b1_b) * silu(...)
                    nc.vector.scalar_tensor_tensor(
                        out=gt[:, goff + fp * T + coff : goff + fp * T + coff + cw],
                        in0=pb[:],
                        scalar=b1sb[e][:, FP + fp : FP + fp + 1],
                        in1=sa[:],
                        op0=ALU.add,
                        op1=ALU.mult,
                    )
                # deferred next-slot x load, off the startup bandwidth peak
                if fp == 6 and e + 1 < EL:
                    nxoff = KD * offs[e + 1]
                    nc.scalar.dma_start(
                        xsb[:, nxoff : nxoff + KD * ts[e + 1]],
                        xt[:, nxoff : nxoff + KD * ts[e + 1]],
                    )
                # slot 0's own W2: paced k-pair loads across its whole
                # GEMM1 (~78 GB/s, gentle enough to never stall W1)
                if e == 0 and fp % 2 == 1:
                    w2_load(0, fp - 1, fp + 1)

            # GEMM2: yT[d 128, tok chunk] = sum_k W2[h_k, d].T @ g[h_k, tok]
            # while it runs, stream the NEXT slot's W2 on scalar (the w2p
            # ring has 2 buffers) and let the w1p ring prefetch the next
            # slot's W1 on sync.
            nsteps = len(chs) * KD
            if e + 1 < EL:
                w2sb[e + 1] = w2p.tile(
                    [128, KH * D], bf16, tag="w2", name=f"w2sb{e + 1}"
                )
            step = 0
            for coff, cw in chs:
                for dt in range(KD):
                    pt = p2.tile([128, cw], f32, tag="p2")
                    for k in range(KH):
                        nc.tensor.matmul(
                            pt[:],
                            lhsT=w2sb[e][:, k * D + dt * 128 : k * D + (dt + 1) * 128],
                            rhs=gt[:, goff + k * T + coff : goff + k * T + coff + cw],
                            start=(k == 0),
                            stop=(k == KH - 1),
                        )
                    ysb = yp.tile([128, cw], bf16, tag="y")
                    nc.scalar.activation(ysb[:], pt[:], AF.Copy, bias=0.0, scale=1.0)
                    nc.scalar.dma_start(
                        y[dt * 128 : (dt + 1) * 128, yoff + coff : yoff + coff + cw],
                        ysb[:],
                    )
                    if e + 1 < EL:
                        klo = (12 * step) // nsteps * 2
                        khi = (12 * (step + 1)) // nsteps * 2
                        if khi > klo:
                            w2_load(e + 1, klo, khi)
                    step += 1
    nc.compile()
    return nc


def _get_nc(ts):
    nc = _NC_CACHE.get(ts)
    if nc is None:
        nc = _NC_CACHE[ts] = _build_nc(ts)
    return nc


def _reorder_weights(W1, W2, b1):
    key = (W1.__array_interface__["data"][0], W2.__array_interface__["data"][0])
    hit = _WCACHE.get(key)
    if hit is not None:
        return hit
    import ml_dtypes

    W1 = np.ascontiguousarray(W1, dtype=np.float32)
    W2 = np.ascontiguousarray(W2, dtype=np.float32)
    b1 = np.ascontiguousarray(b1, dtype=np.float32)
    # W1 [E, D, 2H] -> [E, FB, 128p(d within k), KD*128(f)]
    w1f = (
        W1.reshape(E, KD, 128, FB, 128)
        .transpose(0, 3, 2, 1, 4)
        .reshape(E, FB, 128, KD * 128)
        .astype(ml_dtypes.bfloat16)
    )
    # combine swiglu pair (fp, fp+FP) into one contiguous block per DMA
    w1r = np.ascontiguousarray(np.concatenate([w1f[:, :FP], w1f[:, FP:]], axis=-1))
    # W2 [E, H, D] -> [E, 128p(h within k), KH*D]
    w2t = np.ascontiguousarray(
        W2.reshape(E, KH, 128, D)
        .transpose(0, 2, 1, 3)
        .reshape(E, 128, KH * D)
        .astype(ml_dtypes.bfloat16)
    )
    # b1 [E, 2H] -> [E, 128, FB]
    b1t = np.ascontiguousarray(b1.reshape(E, FB, 128).transpose(0, 2, 1))
    out = (w1r, w2t, b1t)
    _WCACHE.clear()
    _WCACHE[key] = out
    return out


def _route(x_flat, Wr):
    logits = x_flat @ np.ascontiguousarray(Wr, dtype=np.float32)  # [N, E]
    lmax = logits.max(axis=-1, keepdims=True)
    p = np.exp(logits - lmax)
    gates = p / p.sum(axis=-1, keepdims=True)
    expert = np.argmax(gates, axis=-1)
    # slot = occurrence index of each token within its expert's queue
    order = np.argsort(expert, kind="stable")
    sorted_e = expert[order]
    starts = np.searchsorted(sorted_e, np.arange(E))
    within = np.arange(N) - starts[sorted_e]
    slot = np.empty(N, np.int64)
    slot[order] = within
    kept = slot < C
    top_idx = np.zeros((C, E), np.int32)
    valid = np.zeros((C, E), np.float32)
    tok = np.arange(N, dtype=np.int32)
    top_idx[slot[kept], expert[kept]] = tok[kept]
    valid[slot[kept], expert[kept]] = 1.0
    w_ce = gates[top_idx, np.arange(E)[None, :]].astype(np.float32) * valid  # [C, E]
    n_kept = np.minimum(np.bincount(expert, minlength=E), C)  # [E]
    return gates, expert, kept, top_idx, valid, w_ce, n_kept


def _r16(n):
    return max(16, int(n))


def _plan(n_kept):
    """Choose slot sizes + per-(core,slot) expert piece assignment.

    Returns (ts, cores) where cores[c] is a list of (expert, lo, hi)
    token-range pieces, one per slot.
    """
    order = np.argsort(-n_kept, kind="stable")
    # Plan A (2 slots): pair light+heavy; sizes = (8th largest, largest).
    assign2 = [(int(order[E - 1 - i]), int(order[i])) for i in range(NCORES)]
    ts2 = (
        _r16(max(n_kept[a] for a, _ in assign2)),
        _r16(max(n_kept[b] for _, b in assign2)),
    )
    cores2 = [
        [(a, 0, int(n_kept[a])), (b, 0, int(n_kept[b]))] for a, b in assign2
    ]
    # Plan B (3 slots): 8 smallest whole; quartiles 1,2 split in halves
    # across two cores each.
    G1, G2, rest = order[:4], order[4:8], order[8:]
    sC = _r16(n_kept[rest[0]])
    sA = _r16(-(-int(n_kept[G1[0]]) // 2))
    sB = _r16(-(-int(n_kept[G2[0]]) // 2))
    ts3 = (sC, sA, sB)
    cores3 = []
    for c in range(NCORES):
        ec = int(rest[c])
        slots = [(ec, 0, int(n_kept[ec]))]
        for G in (G1, G2):
            e = int(G[c % 4])
            n = int(n_kept[e])
            h = -(-n // 2)
            slots.append((e, 0, h) if c < 4 else (e, h, n))
        cores3.append(slots)
    if _stream_cost(ts3) < _stream_cost(ts2):
        return ts3, cores3
    return ts2, cores2


def kernel(x, Wr, W1, b1, W2, b2, W1f, b1f, W2f, b2f, _trace=False):
    global LAST
    _ensure_concourse()
    import ml_dtypes
    from concourse.bass_utils import run_bass_kernel_spmd

    x_flat = np.ascontiguousarray(np.asarray(x).reshape(N, D), dtype=np.float32)
    gates, expert, kept, top_idx, valid, w_ce, n_kept = _route(x_flat, np.asarray(Wr))
    w1r, w2t, b1t = _reorder_weights(np.asarray(W1), np.asarray(W2), np.asarray(b1))

    ts, cores = _plan(n_kept)

    nc = _get_nc(ts)
    in_maps = []
    for c in range(NCORES):
        pieces = cores[c]
        # gather + transpose tokens for each slot: [128, KD * T]
        xparts = []
        for s, (e, lo, hi) in enumerate(pieces):
            ids = top_idx[lo:hi, e]
            xg = np.zeros((ts[s], D), np.float32)
            xg[: len(ids)] = x_flat[ids]
            xparts.append(
                xg.reshape(ts[s], KD, 128).transpose(2, 1, 0).reshape(128, KD * ts[s])
            )
        xt_c = np.ascontiguousarray(
            np.concatenate(xparts, axis=1), dtype=ml_dtypes.bfloat16
        )
        el = [e for e, _, _ in pieces]
        in_maps.append(
            {
                "xt": xt_c,
                "w1r": np.ascontiguousarray(w1r[el]),
                "w2t": np.ascontiguousarray(w2t[el]),
                "b1t": np.ascontiguousarray(b1t[el]),
            }
        )
    res = run_bass_kernel_spmd(nc, in_maps, list(range(NCORES)), trace=_trace)
    LAST = res

    # Combine: scatter gate-weighted expert outputs back to token order.
    y_flat = np.zeros((N, D), np.float32)
    b2 = np.asarray(b2)
    add_b2 = bool(np.any(b2))
    offs = [sum(ts[:s]) for s in range(len(ts))]
    for c in range(NCORES):
        yc = np.asarray(res.results[c]["y"], dtype=np.float32)  # [D, tot]
        for s, (e, lo, hi) in enumerate(cores[c]):
            n = hi - lo
            ids = top_idx[lo:hi, e]
            off = offs[s]
            w = w_ce[lo:hi, e]
            y_flat[ids] = (yc[:, off : off + n] * w[None, :]).T
            if add_b2:
                y_flat[ids] += w[:, None] * b2[e]

    # Dense fallback for fully-dropped tokens (rare; none at typical loads).
    dropped = ~kept
    if np.any(dropped):
        xd = x_flat[dropped]
        hf = xd @ np.asarray(W1f) + np.asarray(b1f)
        gf = (hf[:, :H] / (1.0 + np.exp(-hf[:, :H]))) * hf[:, H:]
        y_flat[dropped] += FALLBACK_W * (gf @ np.asarray(W2f) + np.asarray(b2f))

    return y_flat.reshape(B, S, D)
